# revision 18
# baseline (speedup 1.0000x reference)
"""Trainium2 Bass kernel for DenseConv2d via Winograd F(2,3) along H.

Conv2d: input (32,128,56,56) f32, weight (256,128,3,3) f32, bias (256,) f32,
stride 1, pad 1, dilation 1 -> output (32,256,56,56) f32.

Data-parallel over batch across 8 NeuronCores (4 images per core). Per core,
the conv uses 1D Winograd F(2,3) on the H axis (direct on W): each tile of 2
output rows consumes 4 transformed-input row streams x 3 kx shifts = 12
matmul columns per 2 rows instead of direct conv's 9 per row -- a 1.5x cut
in tensor-engine columns (94us -> 63us floor at 2.4GHz).

Host-side layout prep (like the baseline's pad/transpose/cast, plus the
standard Winograd weight transform) builds bf16 row tensors
  D2[r] = x[r] - x[r+2]   A[r] = x[r+1] + x[r+2]   S[r] = x[r+2] - x[r+1]
so the PE reads V0=D2 even rows, V1=A even, V2=S even, V3=D2 odd as strided
rhs views (strided rows are free for matmul APs; on-device DVE runs them at
1 elem/cycle which would eat the whole budget).

Per th-block of 8 tiles: 4 PSUM banks (one per kyw, 448 f32 = 1 bank),
12 matmuls (kyw-major 1,2,3,0 so banks release in refill order), then the
inverse transform out_even = M0+M1+M2+b, out_odd = M1-M2-M3+b obeying the
engine rules (max ONE PSUM operand per DVE/ACT op; GpSimd can't touch PSUM):
  ACT:  t = M1+b, c2 = M2          (PSUM->SBUF lifts, bf16 out)
  Pool: s = t+c2                   (SBUF bf16)
  DVE:  d = t-c2 (bf16 2x), odd = d-M3, even = s+M0 (one PSUM operand each)
Outputs store as bf16 on alternating DMA queues; host upcasts to f32.
The final pass ends in two 2-tile blocks to shorten the drain tail.
"""

import sys

if "/opt/trn_rl_repo" not in sys.path:
    sys.path.insert(0, "/opt/trn_rl_repo")

import numpy as np

N_CORES = 8
N, CI, H, W = 32, 128, 56, 56
CO, KH, KW = 256, 3, 3
NP_CORE = N // N_CORES          # images per core
HP, WP = H + 2, W + 2           # padded spatial dims
COT = CO // 128                 # out-channel tiles of 128
NT = H // 2                     # 28 Winograd H-tiles per image
KYW = 4                         # Winograd taps per tile
N_WARMUP = 6                    # full-width PE warmup matmuls
N_WARMUP_SMALL = 4              # half-width tail warmups

BLOCKS = [(0, 8), (8, 8), (16, 8), (24, 4)]
BLOCKS_LAST = [(0, 8), (8, 8), (16, 8), (24, 2), (26, 2)]

_CACHE = {}


def _build_program():
    import concourse.mybir as mybir
    from concourse import bacc
    from concourse.tile import TileContext

    nc = bacc.Bacc(None, target_bir_lowering=False)

    # v: host-transformed input rows, planes 0=D2, 1=A, 2=S
    v_d = nc.dram_tensor("v", [CI, NP_CORE, 3, H, WP], mybir.dt.bfloat16,
                         kind="ExternalInput")
    w_d = nc.dram_tensor("w", [CI, COT, KYW, KW, 128], mybir.dt.bfloat16,
                         kind="ExternalInput")
    b_d = nc.dram_tensor("b2", [128, COT], mybir.dt.float32,
                         kind="ExternalInput")
    y_d = nc.dram_tensor("y", [COT, 128, NP_CORE, H, W], mybir.dt.bfloat16,
                         kind="ExternalOutput")

    f32 = mybir.dt.float32
    bf16 = mybir.dt.bfloat16

    with TileContext(nc) as tc:
        with (
            tc.tile_pool(name="vin", bufs=1) as vpool,
            tc.tile_pool(name="wpool", bufs=1) as wpool,
            tc.tile_pool(name="bpool", bufs=1) as bpool,
            tc.tile_pool(name="tpool", bufs=2) as tpool,
            tc.tile_pool(name="psum", bufs=8, space="PSUM") as ppool,
            tc.tile_pool(name="out", bufs=4) as opool,
        ):
            # PE warmup on scratch data, concurrent with the first input
            # DMAs: bridges PE-free (post-preamble) to data-ready so the
            # clock-gate window keeps the PE at full speed.
            scratch = vpool.tile([CI, 448], bf16, tag="scratch")
            nc.gpsimd.memset(scratch, 0.0)
            wups = ppool.tile([128, 448], f32, tag="ps")
            for _ in range(N_WARMUP):
                nc.tensor.matmul(wups, scratch[:, 0:128], scratch,
                                 start=True, stop=True)
            for _ in range(N_WARMUP_SMALL):
                nc.tensor.matmul(wups[:, 0:224], scratch[:, 0:128],
                                 scratch[:, 0:224], start=True, stop=True)

            # Weights (G-transformed host-side) + bias.
            wt = wpool.tile([CI, COT, KYW, KW, 128], bf16, tag="wt")
            bt = bpool.tile([128, COT], f32)

            # Input rows; img0 lands as two chunks so block0's rows (0:16)
            # are ready early.
            vt = {}
            for img in range(NP_CORE):
                vt[img] = vpool.tile([CI, 3, H, WP], bf16, tag=f"v{img}",
                                     name=f"v{img}")
            nc.sync.dma_start(out=vt[0][:, :, 0:16, :],
                              in_=v_d[:, 0, :, 0:16, :])
            nc.scalar.dma_start(out=wt[:, 0], in_=w_d[:, 0])
            nc.sync.dma_start(out=vt[0][:, :, 16:H, :],
                              in_=v_d[:, 0, :, 16:H, :])
            nc.scalar.dma_start(out=bt, in_=b_d[:, :])
            nc.scalar.dma_start(out=wt[:, 1], in_=w_d[:, 1])
            for img in range(1, NP_CORE):
                eng = nc.sync if img % 2 else nc.scalar
                eng2 = nc.scalar if img % 2 else nc.sync
                eng.dma_start(out=vt[img][:, :, 0:28, :],
                              in_=v_d[:, img, :, 0:28, :])
                eng2.dma_start(out=vt[img][:, :, 28:H, :],
                               in_=v_d[:, img, :, 28:H, :])

            def v_rhs(img, kyw, th0, nt, kx):
                # V0=D2 even rows, V1=A even, V2=S even, V3=D2 odd
                plane = (0, 1, 2, 0)[kyw]
                r0 = 2 * th0 + (1 if kyw == 3 else 0)
                return vt[img][:, plane, r0:r0 + 2 * nt - 1:2, kx:kx + W]

            store_q = [nc.sync, nc.scalar]
            nstore = 0

            for img in range(NP_CORE):
                for cot in range(COT):
                    last_pass = (img == NP_CORE - 1 and cot == COT - 1)
                    for th0, nt in (BLOCKS_LAST if last_pass else BLOCKS):
                        # Fill: 12 matmuls; kyw order 1,2,3,0 matches the
                        # drain's bank-release order (t frees M1 first,
                        # even frees M0 last).
                        ps = {}
                        for kyw in (1, 2, 3, 0):
                            ps[kyw] = ppool.tile([128, nt, W], f32, tag="ps",
                                                 name=f"ps{kyw}")
                            for kx in range(KW):
                                nc.tensor.matmul(
                                    ps[kyw], wt[:, cot, kyw, kx, :],
                                    v_rhs(img, kyw, th0, nt, kx),
                                    start=(kx == 0), stop=(kx == KW - 1),
                                )

                        # Inverse transform + bias (see module docstring).
                        ot = opool.tile([128, 2 * nt, W], bf16, tag="ot")
                        t = tpool.tile([128, nt, W], bf16, tag="t")
                        c2 = tpool.tile([128, nt, W], bf16, tag="c2")
                        s = tpool.tile([128, nt, W], bf16, tag="s")
                        d = tpool.tile([128, nt, W], bf16, tag="d")
                        nc.scalar.add(t, ps[1], bt[:, cot:cot + 1])
                        nc.scalar.copy(c2, ps[2])
                        nc.vector.tensor_sub(d, t, c2)
                        nc.gpsimd.tensor_add(s, t, c2)
                        nc.vector.tensor_sub(
                            ot[:, 1:2 * nt:2, :], d, ps[3])
                        nc.vector.tensor_add(
                            ot[:, 0:2 * nt:2, :], s, ps[0])

                        store_q[nstore % 2].dma_start(
                            out=y_d[cot, :, img, 2 * th0:2 * (th0 + nt), :],
                            in_=ot)
                        nstore += 1

    nc.compile()
    return nc


def prep_in_maps(input, weight, bias):
    """Host-side layout prep -> one in_map per core."""
    import ml_dtypes

    bf = ml_dtypes.bfloat16
    xp = np.pad(input, ((0, 0), (0, 0), (1, 1), (1, 1))).astype(np.float32)

    # Winograd F(2,3) row combinations (plane 0=D2, 1=A, 2=S), bf16.
    def mk_v(xpc):  # xpc: [n, ci, HP, WP] f32 -> [n, ci, 3, H, WP] bf16
        v = np.empty(xpc.shape[:2] + (3, H, WP), dtype=np.float32)
        v[:, :, 0] = xpc[:, :, 0:H, :] - xpc[:, :, 2:H + 2, :]
        v[:, :, 1] = xpc[:, :, 1:H + 1, :] + xpc[:, :, 2:H + 2, :]
        v[:, :, 2] = xpc[:, :, 2:H + 2, :] - xpc[:, :, 1:H + 1, :]
        return v.astype(bf)

    # weight [co, ci, ky, kx] -> G-transform ky -> [ci, cot, kyw, kx, cop]
    g = weight.astype(np.float32)
    u = np.empty((KYW, CO, CI, KW), dtype=np.float32)
    u[0] = g[:, :, 0, :]
    u[1] = 0.5 * (g[:, :, 0, :] + g[:, :, 1, :] + g[:, :, 2, :])
    u[2] = 0.5 * (g[:, :, 0, :] - g[:, :, 1, :] + g[:, :, 2, :])
    u[3] = g[:, :, 2, :]
    wr = np.ascontiguousarray(
        u.reshape(KYW, COT, 128, CI, KW).transpose(3, 1, 0, 4, 2)
    ).astype(bf)
    b2 = np.ascontiguousarray(bias.reshape(COT, 128).T.astype(np.float32))

    in_maps = []
    for c in range(N_CORES):
        xc = xp[c * NP_CORE:(c + 1) * NP_CORE]
        vc = mk_v(xc)  # [np, ci, 3, H, WP] bf16
        vcc = np.ascontiguousarray(vc.transpose(1, 0, 2, 3, 4))
        in_maps.append({"v": vcc, "w": wr, "b2": b2})
    return in_maps


def kernel(input, weight, bias):
    input = np.asarray(input, dtype=np.float32)
    weight = np.asarray(weight, dtype=np.float32)
    bias = np.asarray(bias, dtype=np.float32)

    if "nc" not in _CACHE:
        _CACHE["nc"] = _build_program()
    nc = _CACHE["nc"]

    from concourse.bass_utils import run_bass_kernel_spmd

    in_maps = prep_in_maps(input, weight, bias)
    res = run_bass_kernel_spmd(nc, in_maps, core_ids=list(range(N_CORES)))

    out = np.empty((N, CO, H, W), dtype=np.float32)
    for c in range(N_CORES):
        y = res.results[c]["y"]  # [COT, 128, NP_CORE, H, W] bf16
        out[c * NP_CORE:(c + 1) * NP_CORE] = (
            y.astype(np.float32).transpose(2, 0, 1, 3, 4)
            .reshape(NP_CORE, CO, H, W))
    return out


# revision 22
# speedup vs baseline: 1.1213x; 1.1213x over previous
"""Trainium2 Bass kernel for DenseConv2d via Winograd F(2,3) along H.

Conv2d: input (32,128,56,56) f32, weight (256,128,3,3) f32, bias (256,) f32,
stride 1, pad 1, dilation 1 -> output (32,256,56,56) f32.

Data-parallel over batch across 8 NeuronCores (4 images per core). Per core,
the conv uses 1D Winograd F(2,3) on the H axis (direct on W): each tile of 2
output rows consumes 4 transformed-input row streams x 3 kx shifts = 12
matmul columns per 2 rows instead of direct conv's 9 per row -- a 1.5x cut
in tensor-engine columns (94us -> 63us floor at 2.4GHz).

Host-side layout prep (like the baseline's pad/transpose/cast, plus the
standard Winograd weight transform) builds bf16 row tensors
  D2[r] = x[r] - x[r+2]   A[r] = x[r+1] + x[r+2]   S[r] = x[r+2] - x[r+1]
so the PE reads V0=D2 even rows, V1=A even, V2=S even, V3=D2 odd as strided
rhs views (strided rows are free for matmul APs; on-device DVE runs them at
1 elem/cycle which would eat the whole budget).

Per th-block of 8 tiles: 4 PSUM banks (one per kyw, 448 f32 = 1 bank),
12 matmuls (kyw-major 1,2,3,0 so banks release in refill order), then the
inverse transform out_even = M0+M1+M2+b, out_odd = M1-M2-M3+b obeying the
engine rules (max ONE PSUM operand per DVE/ACT op; GpSimd can't touch PSUM):
  ACT:  t = M1+b, c2 = M2          (PSUM->SBUF lifts, bf16 out)
  Pool: s = t+c2                   (SBUF bf16)
  DVE:  d = t-c2 (bf16 2x), odd = d-M3, even = s+M0 (one PSUM operand each)
Outputs store as bf16 on alternating DMA queues; host upcasts to f32.
The final pass ends in two 2-tile blocks to shorten the drain tail.
"""

import sys

if "/opt/trn_rl_repo" not in sys.path:
    sys.path.insert(0, "/opt/trn_rl_repo")

import numpy as np

N_CORES = 8
N, CI, H, W = 32, 128, 56, 56
CO, KH, KW = 256, 3, 3
NP_CORE = N // N_CORES          # images per core
HP, WP = H + 2, W + 2           # padded spatial dims
COT = CO // 128                 # out-channel tiles of 128
NT = H // 2                     # 28 Winograd H-tiles per image
KYW = 4                         # Winograd taps per tile
N_WARMUP = 6                    # full-width PE warmup matmuls
N_WARMUP_SMALL = 4              # half-width tail warmups

BLOCKS = [(0, 8), (8, 8), (16, 8), (24, 4)]
BLOCKS_LAST = [(0, 8), (8, 8), (16, 8), (24, 2), (26, 2)]

_CACHE = {}


def _build_program():
    import concourse.mybir as mybir
    from concourse import bacc
    from concourse.tile import TileContext

    nc = bacc.Bacc(None, target_bir_lowering=False)

    # v: host-transformed input rows. Per image, 112 rows: 0:56 = D2
    # (both parities used), 56:84 = A even rows, 84:112 = S even rows
    # (A/S odd rows are never read by the matmuls, so they aren't shipped).
    v_d = nc.dram_tensor("v", [CI, NP_CORE, 2 * H, WP], mybir.dt.bfloat16,
                         kind="ExternalInput")
    w_d = nc.dram_tensor("w", [CI, COT, KYW, KW, 128], mybir.dt.bfloat16,
                         kind="ExternalInput")
    b_d = nc.dram_tensor("b2", [128, COT], mybir.dt.float32,
                         kind="ExternalInput")
    y_d = nc.dram_tensor("y", [COT, 128, NP_CORE, H, W], mybir.dt.bfloat16,
                         kind="ExternalOutput")

    f32 = mybir.dt.float32
    bf16 = mybir.dt.bfloat16

    with TileContext(nc) as tc:
        with (
            tc.tile_pool(name="vin", bufs=1) as vpool,
            tc.tile_pool(name="wpool", bufs=1) as wpool,
            tc.tile_pool(name="bpool", bufs=1) as bpool,
            tc.tile_pool(name="tpool", bufs=2) as tpool,
            tc.tile_pool(name="psum", bufs=8, space="PSUM") as ppool,
            tc.tile_pool(name="out", bufs=4) as opool,
        ):
            # PE warmup on scratch data, concurrent with the first input
            # DMAs: bridges PE-free (post-preamble) to data-ready so the
            # clock-gate window keeps the PE at full speed.
            scratch = vpool.tile([CI, 448], bf16, tag="scratch")
            nc.gpsimd.memset(scratch, 0.0)
            wups = ppool.tile([128, 448], f32, tag="ps")
            for _ in range(N_WARMUP):
                nc.tensor.matmul(wups, scratch[:, 0:128], scratch,
                                 start=True, stop=True)
            for _ in range(N_WARMUP_SMALL):
                nc.tensor.matmul(wups[:, 0:224], scratch[:, 0:128],
                                 scratch[:, 0:224], start=True, stop=True)

            # Weights (G-transformed host-side) + bias.
            wt = wpool.tile([CI, COT, KYW, KW, 128], bf16, tag="wt")
            bt = bpool.tile([128, COT], f32)

            # Input rows. img0 lands in consumption order (kyw 1=A, 2=S,
            # 3/0=D2 per block) as small chunks on both queues so block0
            # can start ~2us in; later images land as two big DMAs each.
            vt = {}
            for img in range(NP_CORE):
                vt[img] = vpool.tile([CI, 2 * H, WP], bf16, tag=f"v{img}",
                                     name=f"v{img}")
            v0 = vt[0]
            nc.scalar.dma_start(out=wt[:, 0], in_=w_d[:, 0])
            nc.sync.dma_start(out=v0[:, 56:64, :], in_=v_d[:, 0, 56:64, :])
            nc.sync.dma_start(out=v0[:, 84:92, :], in_=v_d[:, 0, 84:92, :])
            nc.sync.dma_start(out=v0[:, 0:16, :], in_=v_d[:, 0, 0:16, :])
            nc.scalar.dma_start(out=v0[:, 64:84, :], in_=v_d[:, 0, 64:84, :])
            nc.sync.dma_start(out=v0[:, 92:112, :],
                              in_=v_d[:, 0, 92:112, :])
            nc.scalar.dma_start(out=v0[:, 16:34, :], in_=v_d[:, 0, 16:34, :])
            nc.sync.dma_start(out=v0[:, 34:56, :], in_=v_d[:, 0, 34:56, :])
            nc.scalar.dma_start(out=bt, in_=b_d[:, :])
            nc.scalar.dma_start(out=wt[:, 1], in_=w_d[:, 1])
            for img in range(1, NP_CORE):
                eng = nc.sync if img % 2 else nc.scalar
                eng2 = nc.scalar if img % 2 else nc.sync
                eng.dma_start(out=vt[img][:, 0:56, :],
                              in_=v_d[:, img, 0:56, :])
                eng2.dma_start(out=vt[img][:, 56:112, :],
                               in_=v_d[:, img, 56:112, :])

            def v_rhs(img, kyw, th0, nt, kx):
                # V0=D2 even rows, V1=A (rows 56+th), V2=S (84+th),
                # V3=D2 odd rows
                if kyw == 1 or kyw == 2:
                    r0 = (56 if kyw == 1 else 84) + th0
                    return vt[img][:, r0:r0 + nt, kx:kx + W]
                r0 = 2 * th0 + (1 if kyw == 3 else 0)
                return vt[img][:, r0:r0 + 2 * nt - 1:2, kx:kx + W]

            store_q = [nc.sync, nc.scalar]
            nstore = 0

            for img in range(NP_CORE):
                for cot in range(COT):
                    last_pass = (img == NP_CORE - 1 and cot == COT - 1)
                    for th0, nt in (BLOCKS_LAST if last_pass else BLOCKS):
                        # Fill: 12 matmuls; kyw order 1,2,3,0 matches the
                        # drain's bank-release order (t frees M1 first,
                        # even frees M0 last).
                        ps = {}
                        for kyw in (1, 2, 3, 0):
                            ps[kyw] = ppool.tile([128, nt, W], f32, tag="ps",
                                                 name=f"ps{kyw}")
                            for kx in range(KW):
                                nc.tensor.matmul(
                                    ps[kyw], wt[:, cot, kyw, kx, :],
                                    v_rhs(img, kyw, th0, nt, kx),
                                    start=(kx == 0), stop=(kx == KW - 1),
                                )

                        # Inverse transform + bias (see module docstring).
                        ot = opool.tile([128, 2 * nt, W], bf16, tag="ot")
                        t = tpool.tile([128, nt, W], bf16, tag="t")
                        c2 = tpool.tile([128, nt, W], bf16, tag="c2")
                        s = tpool.tile([128, nt, W], bf16, tag="s")
                        d = tpool.tile([128, nt, W], bf16, tag="d")
                        nc.scalar.add(t, ps[1], bt[:, cot:cot + 1])
                        nc.scalar.copy(c2, ps[2])
                        nc.vector.tensor_sub(d, t, c2)
                        nc.gpsimd.tensor_add(s, t, c2)
                        nc.vector.tensor_sub(
                            ot[:, 1:2 * nt:2, :], d, ps[3])
                        nc.vector.tensor_add(
                            ot[:, 0:2 * nt:2, :], s, ps[0])

                        store_q[nstore % 2].dma_start(
                            out=y_d[cot, :, img, 2 * th0:2 * (th0 + nt), :],
                            in_=ot)
                        nstore += 1

    nc.compile()
    return nc


def prep_in_maps(input, weight, bias):
    """Host-side layout prep -> one in_map per core."""
    import ml_dtypes

    bf = ml_dtypes.bfloat16
    xp = np.pad(input, ((0, 0), (0, 0), (1, 1), (1, 1))).astype(np.float32)

    # Winograd F(2,3) row combinations: rows 0:56 = D2, 56:84 = A even,
    # 84:112 = S even (A/S odd rows unused by the kernel).
    def mk_v(xpc):  # xpc: [n, ci, HP, WP] f32 -> [n, ci, 112, WP] bf16
        v = np.empty(xpc.shape[:2] + (2 * H, WP), dtype=np.float32)
        v[:, :, 0:H] = xpc[:, :, 0:H, :] - xpc[:, :, 2:H + 2, :]
        v[:, :, H:H + NT] = (xpc[:, :, 1:H + 1:2, :]
                             + xpc[:, :, 2:H + 2:2, :])
        v[:, :, H + NT:] = (xpc[:, :, 2:H + 2:2, :]
                            - xpc[:, :, 1:H + 1:2, :])
        return v.astype(bf)

    # weight [co, ci, ky, kx] -> G-transform ky -> [ci, cot, kyw, kx, cop]
    g = weight.astype(np.float32)
    u = np.empty((KYW, CO, CI, KW), dtype=np.float32)
    u[0] = g[:, :, 0, :]
    u[1] = 0.5 * (g[:, :, 0, :] + g[:, :, 1, :] + g[:, :, 2, :])
    u[2] = 0.5 * (g[:, :, 0, :] - g[:, :, 1, :] + g[:, :, 2, :])
    u[3] = g[:, :, 2, :]
    wr = np.ascontiguousarray(
        u.reshape(KYW, COT, 128, CI, KW).transpose(3, 1, 0, 4, 2)
    ).astype(bf)
    b2 = np.ascontiguousarray(bias.reshape(COT, 128).T.astype(np.float32))

    in_maps = []
    for c in range(N_CORES):
        xc = xp[c * NP_CORE:(c + 1) * NP_CORE]
        vc = mk_v(xc)  # [np, ci, 112, WP] bf16
        vcc = np.ascontiguousarray(vc.transpose(1, 0, 2, 3))
        in_maps.append({"v": vcc, "w": wr, "b2": b2})
    return in_maps


def kernel(input, weight, bias):
    input = np.asarray(input, dtype=np.float32)
    weight = np.asarray(weight, dtype=np.float32)
    bias = np.asarray(bias, dtype=np.float32)

    if "nc" not in _CACHE:
        _CACHE["nc"] = _build_program()
    nc = _CACHE["nc"]

    from concourse.bass_utils import run_bass_kernel_spmd

    in_maps = prep_in_maps(input, weight, bias)
    res = run_bass_kernel_spmd(nc, in_maps, core_ids=list(range(N_CORES)))

    out = np.empty((N, CO, H, W), dtype=np.float32)
    for c in range(N_CORES):
        y = res.results[c]["y"]  # [COT, 128, NP_CORE, H, W] bf16
        out[c * NP_CORE:(c + 1) * NP_CORE] = (
            y.astype(np.float32).transpose(2, 0, 1, 3, 4)
            .reshape(NP_CORE, CO, H, W))
    return out
